# revision 1
# baseline (speedup 1.0000x reference)
"""Trainium2 Bass kernel for the 3-expert MoE routing MLP.

Reference computation (B=1M rows):
    y1  = tanh(x @ w1 - b1)                     # [B, 8]
    h_k = sigmoid(y1 @ wa_k - ba_k)             # [B, 16] for experts k=0,1,2
    e_k = h_k @ wb_k - bb_k                     # [B, 32]
    y   = e_{u[b]}  per row b

Device mapping (pure data parallel over 8 cores, B_C = 125000 rows/core):

  * Host packs each core's x shard transposed, two row-blocks deep:
    x2 [128, B_H] fp16 with partitions 0-63 = x[:B_H].T, 64-127 = x[B_H:].T,
    so every PE matmul streams two batch halves at once (B_H = B_C/2).

  * The whole per-row pipeline is 4 fp16 matmuls + ONE sigmoid + one
    int-compare + one PSUM->SBUF copy, using a software-pipelined
    ("skewed") PSUM bank S(c) per 500-column chunk c:

      rows 0-15  : 2*(x@w1 - b1)(c)        <- mmA (lhsT 2*w1 blocks)
      rows 16-21 : BIG*onehot(c)           <- mmB (from DVE is_equal)
      rows 22-27 : BIG*pseudo-onehot(c-1)  <- mmH (propagated)
      rows 32-127: H-preact(c-1) + BIG*mask<- mmH (from T(c-1) rows 0:22)

    One ACT sigmoid over S(c) then yields T(c) fp16:
      rows 0-15  : y1t'(c)   = sigmoid(2*(x@w1-b1))      (tanh = 2*sig-1)
      rows 16-21 : poh(c)    ~= onehot(c)    (sigmoid saturation)
      rows 22-27 : poh(c-1)  ~= onehot(c-1)
      rows 32-127: G(c-1)    = sigmoid(P_h - ba - BIG(1-mask)) ~= mask*h

    mmF(c-1) then computes the final output in one stream from T(c):
      lhsT rows 22-27 = -bb_k rows (bias select via poh(c-1)),
      lhsT rows 32-127 = block-diag wb (the tanh "-1" constant is folded
      into the sigmoid bias as -colsum(wa)).

  * Output is written transposed ([64, B_H] f32 per core) for contiguous
    DMA; the host unpacks back to [B, 32].
"""

import math

import numpy as np

import concourse.bass as bass
import concourse.tile as tile
from concourse import mybir
from concourse.bass_utils import run_bass_kernel_spmd

F32 = mybir.dt.float32
F16 = mybir.dt.float16
I32 = mybir.dt.int32

N_CORES = 8
B = 1_000_000
IN = 64
OUT = 32
B_C = B // N_CORES          # rows per core
B_H = B_C // 2              # packed free length per core
BIG = 28.0                  # saturation constant (sigmoid(-14) ~ 8e-7)

F_TILE = 2000               # SBUF tile free size (4 chunks)
CHUNK = 500                 # PSUM matmul free size (<=512 fp32 out)


def _pack_weights(w1, b1, w2, b2, w3, b3, w4, b4, w5, b5, w6, b6, w7, b7):
    f32 = np.float32
    # mmA lhsT [128, 16]: 2*w1 block-diag (tanh(z) = 2*sigmoid(2z) - 1).
    W_a = np.zeros((128, 16), f32)
    W_a[0:64, 0:8] = 2.0 * w1
    W_a[64:128, 8:16] = 2.0 * w1

    # mmH lhsT [22, 96]: K-rows = T_y[0:22] = [y1t'(16); onehot(6)].
    #   y1t' rows: 2*wa blocks (y1 = 2*y1t' - 1)
    #   onehot rows: +BIG on the selected expert's 16 hidden rows
    wa_all = np.concatenate([w2, w4, w6], axis=1)        # [8, 48]
    W_h = np.zeros((22, 96), f32)
    W_h[0:8, 0:48] = 2.0 * wa_all
    W_h[8:16, 48:96] = 2.0 * wa_all
    for k in range(3):
        W_h[16 + k, 16 * k:16 * (k + 1)] = BIG
        W_h[19 + k, 48 + 16 * k:64 + 16 * k] = BIG

    # mmF lhsT [102, 64]: rows 0-95 wb blocks, rows 96-101 -bb (onehot).
    wb_all = np.concatenate([w3, w5, w7], axis=0)        # [48, 32]
    bb = [b3, b5, b7]
    W_f = np.zeros((102, 64), f32)
    W_f[0:48, 0:32] = wb_all
    W_f[48:96, 32:64] = wb_all
    for k in range(3):
        W_f[96 + k, 0:32] = -bb[k]
        W_f[99 + k, 32:64] = -bb[k]

    # sigmoid biases: trunk -2*b1 ; H: -ba - BIG - colsum(wa) (the "-1"
    # of y1 = 2*y1t' - 1 folded in).
    ba_all = np.concatenate([b2, b4, b6])                # [48]
    colsum = wa_all.sum(axis=0)                          # [48]
    b1v = np.concatenate([-2.0 * b1, -2.0 * b1]).astype(f32)
    b2v = np.concatenate([-ba_all - BIG - colsum] * 2).astype(f32)

    # bpack [102, 3]: col0 rows0-15 = b1v, col1 rows0-95 = b2v,
    # col2 rows0-5 = kvec.
    bpack = np.zeros((102, 3), f32)
    bpack[0:16, 0] = b1v
    bpack[0:96, 1] = b2v
    bpack[0:6, 2] = np.array([0, 1, 2, 0, 1, 2], f32)

    # wpack fp16 [128, 176]: cols 0:16 W_a, 16:112 W_h, 112:176 W_f.
    wpack = np.zeros((128, 176), np.float16)
    wpack[:, 0:16] = W_a.astype(np.float16)
    wpack[0:22, 16:112] = W_h.astype(np.float16)
    wpack[0:102, 112:176] = W_f.astype(np.float16)
    return dict(wpack=wpack, bpack=bpack)


def _split_multi_waits(nc):
    """Walrus codegen allows one sync-wait per instruction; hoist extra
    waits onto same-engine NoOps inserted just before the instruction."""
    n = 0
    for fn in nc.m.functions:
        for blk in fn.blocks:
            out = []
            for ins in blk.instructions:
                si = ins.sync_info
                if si is not None and len(si.on_wait) > 1:
                    waits = list(si.on_wait)
                    for j, w in enumerate(waits[:-1]):
                        nop = mybir.InstNoOp(name=f"{ins.name}-wsplit{j}")
                        nop.engine = ins.engine
                        nop.sync_info = mybir.SyncInfo(on_wait=[w],
                                                       on_update=[])
                        nc.register_instruction(nop)
                        out.append(nop)
                        n += 1
                    si.on_wait = [waits[-1]]
                out.append(ins)
            blk.instructions[:] = out
    return n


def build_nc(b_h=B_H, f_tile=F_TILE, chunk=CHUNK):
    nc = bass.Bass("TRN2", target_bir_lowering=False, debug=False)

    x2_d = nc.dram_tensor("x2", [128, b_h], F16, kind="ExternalInput").ap()
    u6_d = nc.dram_tensor("u6", [6, b_h], I32, kind="ExternalInput").ap()
    wp_d = nc.dram_tensor("wpack", [128, 176], F16, kind="ExternalInput").ap()
    bp_d = nc.dram_tensor("bpack", [102, 3], F32, kind="ExternalInput").ap()
    yt_d = nc.dram_tensor("yT", [64, b_h], F32, kind="ExternalOutput").ap()

    assert b_h % chunk == 0 and f_tile % chunk == 0
    n_tiles = math.ceil(b_h / f_tile)

    SIG = mybir.ActivationFunctionType.Sigmoid

    with tile.TileContext(nc) as tc:
        with (
            tc.tile_pool(name="const", bufs=1) as cpool,
            tc.tile_pool(name="xin", bufs=4) as xpool,
            tc.tile_pool(name="uin", bufs=3) as upool,
            tc.tile_pool(name="toh", bufs=2) as ohpool,
            tc.tile_pool(name="ty", bufs=2) as ypool,
            tc.tile_pool(name="tg", bufs=2) as gpool,
            tc.tile_pool(name="outp", bufs=3) as opool,
            tc.tile_pool(name="st", bufs=2, space="PSUM") as stp,
            tc.tile_pool(name="sh", bufs=3, space="PSUM") as shp,
            tc.tile_pool(name="pfp", bufs=3, space="PSUM") as pfp,
        ):
            wp = cpool.tile([128, 176], F16)
            nc.sync.dma_start(wp[:], wp_d)
            W_a = wp[:, 0:16]
            W_h = wp[0:22, 16:112]
            W_f = wp[0:102, 112:176]
            bp = cpool.tile([102, 3], F32)
            nc.sync.dma_start(bp[:], bp_d)
            b1v = bp[0:16, 0:1]
            b2v = bp[0:96, 1:2]
            kv = bp[0:6, 2:3]

            # HAM warmup: ~8us of dependency-free back-to-back matmuls so
            # the PE clock gate opens (4/8 -> 8/8) before the main loop.
            for _ in range(28):
                wps = stp.tile([16, chunk], F32, name="wps", tag="S_t")
                nc.tensor.matmul(wps[:, 0:176], wp[:, 0:16], wp[:, 0:176],
                                 start=True, stop=True)

            n_chunks = b_h // chunk
            xt = {}; ut = {}; oh = {}; T_y = {}; T_g = {}; ot = {}
            S_t = {}; S_h = {}; p_f = {}
            cpt = f_tile // chunk

            def colsl(c):
                return slice((c % cpt) * chunk, (c % cpt) * chunk + chunk)

            # Software-pipelined emission: per step s the PE does
            # mmA(s), mmH(s-1), mmF(s-2) so every PE wait is pre-satisfied
            # and the PE stream stays dense.
            for s in range(n_chunks + 2):
                c0, c1, c2 = s, s - 1, s - 2
                if c0 < n_chunks:
                    t = c0 // cpt
                    if c0 % cpt == 0:
                        f0 = t * f_tile
                        fs = min(f_tile, b_h - f0)
                        xt[t] = xpool.tile([128, f_tile], F16, name="xt")
                        nc.sync.dma_start(xt[t][:, :fs], x2_d[:, f0:f0 + fs])
                        ut[t] = upool.tile([6, f_tile], I32, name="ut")
                        nc.sync.dma_start(ut[t][:, :fs], u6_d[:, f0:f0 + fs])
                        oh[t] = ohpool.tile([6, f_tile], F16, name="oh")
                        nc.vector.tensor_scalar(
                            oh[t][:, :fs], ut[t][:, :fs], kv[:], None,
                            mybir.AluOpType.is_equal)
                        T_y[t] = ypool.tile([22, f_tile], F16, name="T_y")
                        T_g[t] = gpool.tile([102, f_tile], F16, name="T_g")
                        nc.sync.dma_start(T_y[t][16:22, :fs], oh[t][:, :fs])
                        nc.sync.dma_start(T_g[t][96:102, :fs], oh[t][:, :fs])
                        ot[t] = opool.tile([64, f_tile], F32, name="ot")
                    cc = colsl(c0)
                    S_t[c0] = stp.tile([16, chunk], F32, name="S_t")
                    nc.tensor.matmul(S_t[c0][:], W_a, xt[t][:, cc],
                                     start=True, stop=True)
                    nc.scalar.activation(T_y[t][0:16, cc], S_t[c0][:], SIG,
                                         bias=b1v[:], scale=1.0)
                    del S_t[c0]
                if 0 <= c2 < n_chunks:
                    t2 = c2 // cpt
                    cc = colsl(c2)
                    pf = pfp.tile([64, chunk], F32, name="pf")
                    nc.tensor.matmul(pf[:], W_f, T_g[t2][0:102, cc],
                                     start=True, stop=True)
                    nc.vector.tensor_copy(ot[t2][:, cc], pf[:])
                    if c2 % cpt == cpt - 1 or c2 == n_chunks - 1:
                        f0 = t2 * f_tile
                        ofs = min(f_tile, b_h - f0)
                        nc.gpsimd.dma_start(yt_d[:, f0:f0 + ofs],
                                            ot[t2][:, :ofs])
                if 0 <= c1 < n_chunks:
                    t1 = c1 // cpt
                    cc = colsl(c1)
                    S_h[c1] = shp.tile([96, chunk], F32, name="S_h")
                    nc.tensor.matmul(S_h[c1][:], W_h, T_y[t1][0:22, cc],
                                     start=True, stop=True)
                    nc.scalar.activation(T_g[t1][0:96, cc], S_h[c1][:], SIG,
                                         bias=b2v[:], scale=1.0)
                    del S_h[c1]

    _split_multi_waits(nc)
    return nc


_NC_CACHE = {}


def _get_nc(b_h=B_H, f_tile=F_TILE, chunk=CHUNK):
    key = (b_h, f_tile, chunk)
    if key not in _NC_CACHE:
        _NC_CACHE[key] = build_nc(*key)
    return _NC_CACHE[key]


def make_in_maps(x, u, weights, n_cores=N_CORES):
    """Shard + pack full inputs into per-core in_maps."""
    packed = _pack_weights(*weights)
    b = x.shape[0]
    b_c = b // n_cores
    b_h = b_c // 2
    in_maps = []
    for c in range(n_cores):
        xc = x[c * b_c:(c + 1) * b_c]
        uc = u[c * b_c:(c + 1) * b_c]
        x2 = np.empty((128, b_h), np.float16)
        x2[0:64] = xc[:b_h].T
        x2[64:128] = xc[b_h:].T
        u6 = np.empty((6, b_h), np.int32)
        u6[0:3] = uc[:b_h]
        u6[3:6] = uc[b_h:]
        in_maps.append({"x2": x2, "u6": u6, **packed})
    return in_maps


def unpack_outputs(results, n_cores=N_CORES):
    b_h = results[0]["yT"].shape[1]
    b_c = 2 * b_h
    y = np.empty((n_cores * b_c, OUT), np.float32)
    for c in range(n_cores):
        yt = results[c]["yT"]
        y[c * b_c:c * b_c + b_h] = yt[0:32].T
        y[c * b_c + b_h:(c + 1) * b_c] = yt[32:64].T
    return y


def kernel(x, u, w1, b1, w2, b2, w3, b3, w4, b4, w5, b5, w6, b6, w7, b7):
    x = np.asarray(x, np.float32)
    u = np.ascontiguousarray(np.asarray(u, np.int32))
    weights = [np.asarray(t, np.float32) for t in
               (w1, b1, w2, b2, w3, b3, w4, b4, w5, b5, w6, b6, w7, b7)]

    nc = _get_nc()
    in_maps = make_in_maps(x, u, weights)
    res = run_bass_kernel_spmd(nc, in_maps, core_ids=list(range(N_CORES)))
    return unpack_outputs(res.results)



# revision 10
# speedup vs baseline: 2.3946x; 2.3946x over previous
"""Trainium2 Bass kernel for the 3-expert MoE routing MLP (expert-sorted).

Reference computation (B=1M rows):
    y1  = tanh(x @ w1 - b1)                     # [B, 8]
    h_k = sigmoid(y1 @ wa_k - ba_k)             # [B, 16] for experts k=0,1,2
    e_k = h_k @ wb_k - bb_k                     # [B, 32]
    y   = e_{u[b]}  per row b

Strategy: the HOST does the routing.  Rows are sorted by expert id and
packed into a fixed per-core layout: each of the 8 cores gets 63000
columns (2 rows per column: "top" features in partitions 0:64, "bottom"
in 64:128), where columns [e*21000, (e+1)*21000) hold only expert-e rows
(zero-padded; counts are ~41.7k of a 42k quota per core per expert).
The device then runs a dense per-expert MLP with NO masking/gather:

Per quad (4 chunks of 375 columns, single expert by construction):
  * 4x mmA    : [128,16]  x2-chunk -> psA[16k:16k+16]   (2*w1 blocks)
  * 1x ACT    : sigmoid(psA[0:64] - 2*b1) -> T_y4       (tanh via 2sig-1)
  * 1x mmH4   : [64,128] block-diag 2*wa_e over 4 chunks -> psH[0:128]
  * 1x ACT    : sigmoid(psH - ba_e - colsum(wa_e)) -> T_h
  * 2x mmF2   : [64,128] block-diag wb_e over 2 chunks -> psF0/psF1
  * copy+bias : DVE (psF0) and GPSIMD (psF1) tensor_scalar add of -bb_e,
                f32 PSUM -> fp16 SBUF, then one DMA out per quad.

This packs the tiny per-row matmuls across chunks in the partition dim:
1.75 PE cycles/column (vs 3.0 for the mask-based kernel), 2 activation
instructions per 1500 columns (vs 12), and an fp16 output stream.
The host inverts the permutation on the way out.
"""

import math

import numpy as np

import concourse.bass as bass
import concourse.tile as tile
from concourse import mybir
from concourse.bass_utils import run_bass_kernel_spmd

F32 = mybir.dt.float32
F16 = mybir.dt.float16

N_CORES = 8
B = 1_000_000
IN = 64
OUT = 32

CHUNK = 375                  # columns per PSUM tile (375*4B <= 2KB bank)
M_QUADS = 14                 # quads per expert per core
Q_COLS = 4 * CHUNK * M_QUADS     # 21000 columns per expert per core
NQ = 3 * M_QUADS             # 42 quads per core
B_H = 3 * Q_COLS             # 63000 columns per core
CAP = 2 * Q_COLS             # 42000 rows per expert per core


def _pack_weights(w1, b1, w2, b2, w3, b3, w4, b4, w5, b5, w6, b6, w7, b7):
    f32 = np.float32
    was = [w2, w4, w6]
    bas = [b2, b4, b6]
    wbs = [w3, w5, w7]
    bbs = [b3, b5, b7]

    # mmA lhsT [128, 32]: 2*w1 half-blocks in cols 0:16, zeros in cols
    # 16:32 (PSUM matmul writes must be 32-row aligned, so each chunk's
    # 16 trunk rows are padded to a 32-row group with explicit zeros --
    # PE cost only depends on the moving free size, not output rows).
    W_a = np.zeros((128, 32), f32)
    W_a[0:64, 0:8] = 2.0 * w1
    W_a[64:128, 8:16] = 2.0 * w1

    wpack = np.zeros((128, 32 + 3 * 128 + 3 * 128), np.float16)
    wpack[:, 0:32] = W_a.astype(np.float16)

    # mmH4 lhsT [128, 128] per expert: 4 chunk blocks of [32 -> 32];
    # block k: rows 32k:32k+8 (y1t top) -> cols 32k:32k+16 (h top),
    # rows 32k+8:32k+16 (y1t bottom) -> cols 32k+16:32k+32; rows
    # 32k+16:32k+32 are the zero-pad rows (zero weights).
    for e in range(3):
        W_h = np.zeros((128, 128), f32)
        for k in range(4):
            r, c = 32 * k, 32 * k
            W_h[r:r + 8, c:c + 16] = 2.0 * was[e]
            W_h[r + 8:r + 16, c + 16:c + 32] = 2.0 * was[e]
        wpack[:, 32 + 128 * e:32 + 128 * (e + 1)] = W_h.astype(np.float16)

    # mmF2 lhsT [64, 128] per expert: 2 chunk blocks of [32 -> 64],
    # each block 2 half-blocks of wb_e [16, 32].  Stored twice (rows
    # 0:64 and 64:128) because PE weights must sit on the same SBUF
    # partitions as the moving operand (th[0:64] / th[64:128]).
    for e in range(3):
        W_f = np.zeros((64, 128), f32)
        for j in range(2):
            r, c = 32 * j, 64 * j
            W_f[r:r + 16, c:c + 32] = wbs[e]
            W_f[r + 16:r + 32, c + 32:c + 64] = wbs[e]
        wf16 = W_f.astype(np.float16)
        wpack[0:64, 416 + 128 * e:416 + 128 * (e + 1)] = wf16
        wpack[64:128, 416 + 128 * e:416 + 128 * (e + 1)] = wf16

    # bpack [128, 7] f32: col 0 = trunk sigmoid bias (-2*b1 per half,
    # tiled over 4 chunk blocks); cols 1..3 = hidden sigmoid bias per
    # expert (-ba_e - colsum(wa_e), the tanh "-1" folded in); cols 4..6 =
    # output bias per expert (-bb_e tiled over chunk/half blocks).
    bpack = np.zeros((128, 7), f32)
    blkA = np.concatenate([-2.0 * b1, -2.0 * b1, np.zeros(16, f32)])  # [32]
    bpack[:, 0] = np.tile(blkA, 4)
    for e in range(3):
        hv = -bas[e] - was[e].sum(axis=0)                    # [16]
        bpack[:, 1 + e] = np.tile(np.concatenate([hv, hv]), 4)
        bpack[:, 4 + e] = np.tile(-bbs[e], 4)
    return dict(wpack=wpack, bpack=bpack)


def _split_multi_waits(nc):
    """Walrus codegen allows one sync-wait per instruction; hoist extra
    waits onto same-engine NoOps inserted just before the instruction."""
    n = 0
    for fn in nc.m.functions:
        for blk in fn.blocks:
            out = []
            for ins in blk.instructions:
                si = ins.sync_info
                if si is not None and len(si.on_wait) > 1:
                    waits = list(si.on_wait)
                    for j, w in enumerate(waits[:-1]):
                        nop = mybir.InstNoOp(name=f"{ins.name}-wsplit{j}")
                        nop.engine = ins.engine
                        nop.sync_info = mybir.SyncInfo(on_wait=[w],
                                                       on_update=[])
                        nc.register_instruction(nop)
                        out.append(nop)
                        n += 1
                    si.on_wait = [waits[-1]]
                out.append(ins)
            blk.instructions[:] = out
    return n


def build_nc(chunk=CHUNK, m_quads=M_QUADS):
    nc = bass.Bass("TRN2", target_bir_lowering=False, debug=False)

    nq = 3 * m_quads
    b_h = 4 * chunk * nq
    quad = 4 * chunk

    x2_d = nc.dram_tensor("x2", [128, b_h], F16, kind="ExternalInput").ap()
    wp_d = nc.dram_tensor("wpack", [128, 800], F16, kind="ExternalInput").ap()
    bp_d = nc.dram_tensor("bpack", [128, 7], F32, kind="ExternalInput").ap()
    yt_d = nc.dram_tensor("yT", [128, b_h // 2], F16, kind="ExternalOutput").ap()

    SIG = mybir.ActivationFunctionType.Sigmoid
    ADD = mybir.AluOpType.add

    with tile.TileContext(nc) as tc:
        with (
            tc.tile_pool(name="const", bufs=1) as cpool,
            tc.tile_pool(name="xin", bufs=4) as xpool,
            tc.tile_pool(name="ty", bufs=2) as typool,
            tc.tile_pool(name="th", bufs=2) as thpool,
            tc.tile_pool(name="outp", bufs=3) as opool,
            tc.tile_pool(name="psa", bufs=2, space="PSUM") as psapool,
            tc.tile_pool(name="psh", bufs=2, space="PSUM") as pshpool,
            tc.tile_pool(name="psf", bufs=2, space="PSUM") as psfpool,
        ):
            wp = cpool.tile([128, 800], F16)
            nc.sync.dma_start(wp[:], wp_d)
            bp = cpool.tile([128, 7], F32)
            nc.sync.dma_start(bp[:], bp_d)

            def W_h4(e):
                return wp[:, 32 + 128 * e:32 + 128 * (e + 1)]

            def W_f2(e, j):
                return wp[64 * j:64 * (j + 1),
                          416 + 128 * e:416 + 128 * (e + 1)]

            xt = {}
            ty = {}
            th = {}

            def fetch_x(q):
                if q < nq:
                    xt[q] = xpool.tile([128, quad], F16, name="xt")
                    nc.sync.dma_start(xt[q][:], x2_d[:, q * quad:(q + 1) * quad])

            fetch_x(0)
            fetch_x(1)

            # 3-stage software pipeline over quads: per step s the PE runs
            # mmA(s) x4, mmH4(s-1), mmF2(s-2) x2 so its waits are
            # pre-satisfied and the stream stays dense.
            for s in range(nq + 2):
                q0, q1, q2 = s, s - 1, s - 2
                if q0 < nq:
                    fetch_x(q0 + 2)
                    psa = psapool.tile([128, chunk], F32, name="psa")
                    for k in range(4):
                        nc.tensor.matmul(
                            psa[32 * k:32 * (k + 1), :], wp[:, 0:32],
                            xt[q0][:, k * chunk:(k + 1) * chunk],
                            start=True, stop=True,
                            tile_position=(0, 32 * k))
                    ty[q0] = typool.tile([128, chunk], F16, name="ty")
                    nc.scalar.activation(ty[q0][:], psa[:], SIG,
                                         bias=bp[0:128, 0:1], scale=1.0)
                    del xt[q0]
                if 0 <= q1 < nq:
                    e1 = q1 // m_quads
                    psh = pshpool.tile([128, chunk], F32, name="psh")
                    nc.tensor.matmul(psh[:], W_h4(e1), ty[q1][:],
                                     start=True, stop=True)
                    th[q1] = thpool.tile([128, chunk], F16, name="th")
                    nc.scalar.activation(th[q1][:], psh[:], SIG,
                                         bias=bp[0:128, 1 + e1:2 + e1],
                                         scale=1.0)
                    del ty[q1]
                if 0 <= q2 < nq:
                    e2 = q2 // m_quads
                    psf0 = psfpool.tile([128, chunk], F32, name="psf0")
                    nc.tensor.matmul(psf0[:], W_f2(e2, 0), th[q2][0:64, :],
                                     start=True, stop=True)
                    psf1 = psfpool.tile([128, chunk], F32, name="psf1")
                    nc.tensor.matmul(psf1[:], W_f2(e2, 1), th[q2][64:128, :],
                                     start=True, stop=True)
                    ot = opool.tile([128, 2 * chunk], F16, name="ot")
                    bb = bp[0:128, 4 + e2:5 + e2]
                    nc.vector.tensor_scalar(ot[:, 0:chunk], psf0[:], bb, None,
                                            ADD)
                    nc.vector.tensor_scalar(ot[:, chunk:2 * chunk], psf1[:],
                                            bb, None, ADD)
                    nc.gpsimd.dma_start(
                        yt_d[:, q2 * 2 * chunk:(q2 + 1) * 2 * chunk], ot[:])
                    del th[q2]

    _split_multi_waits(nc)
    return nc


_NC_CACHE = {}


def _get_nc(chunk=CHUNK, m_quads=M_QUADS):
    key = (chunk, m_quads)
    if key not in _NC_CACHE:
        _NC_CACHE[key] = build_nc(*key)
    return _NC_CACHE[key]


_PACK_STATE = {}


def make_in_maps(x, u, weights, n_cores=N_CORES):
    """Sort rows by expert, pack into per-core fixed-quota layouts."""
    packed = _pack_weights(*weights)
    b = x.shape[0]

    order = np.argsort(u, kind="stable")
    counts = np.bincount(u, minlength=3)
    if counts.max() > n_cores * CAP:
        _PACK_STATE["fallback"] = (x, u, weights)
        return None

    # top_idx/bot_idx [n_cores, B_H]: source row per packed slot, -1 = pad.
    top_idx = np.full((n_cores, B_H), -1, np.int64)
    bot_idx = np.full((n_cores, B_H), -1, np.int64)
    start = 0
    for e in range(3):
        idx_e = order[start:start + counts[e]]
        start += counts[e]
        parts = np.array_split(idx_e, n_cores)
        for c in range(n_cores):
            p = parts[c]
            col0 = e * Q_COLS
            n_top = min(len(p), Q_COLS)
            top_idx[c, col0:col0 + n_top] = p[:n_top]
            n_bot = len(p) - n_top
            if n_bot > 0:
                bot_idx[c, col0:col0 + n_bot] = p[n_top:]

    xh = x.astype(np.float16)
    in_maps = []
    for c in range(n_cores):
        x2 = np.zeros((128, B_H), np.float16)
        tv = top_idx[c] >= 0
        bv = bot_idx[c] >= 0
        x2[0:64, tv] = xh[top_idx[c][tv]].T
        x2[64:128, bv] = xh[bot_idx[c][bv]].T
        in_maps.append({"x2": x2, **packed})

    _PACK_STATE["fallback"] = None
    _PACK_STATE["top_idx"] = top_idx
    _PACK_STATE["bot_idx"] = bot_idx
    return in_maps


def _numpy_reference(x, u, weights):
    w1, b1, w2, b2, w3, b3, w4, b4, w5, b5, w6, b6, w7, b7 = weights
    y1 = np.tanh(x @ w1 - b1)
    out = np.zeros((x.shape[0], OUT), np.float32)
    for e, (wa, ba, wb, bb) in enumerate(
            [(w2, b2, w3, b3), (w4, b4, w5, b5), (w6, b6, w7, b7)]):
        m = u == e
        h = 1.0 / (1.0 + np.exp(-(y1[m] @ wa - ba)))
        out[m] = h @ wb - bb
    return out


def unpack_outputs(results, n_cores=N_CORES):
    top_idx = _PACK_STATE["top_idx"]
    bot_idx = _PACK_STATE["bot_idx"]
    y = np.empty((B, OUT), np.float32)
    for c in range(n_cores):
        yt = results[c]["yT"]  # [128, B_H/2] fp16
        # rows: (chunk-in-pair, half, feat); cols: (quad, pair, col375)
        arr = yt.reshape(2, 2, 32, NQ, 2, CHUNK)
        # -> (half, quad, pair, cip, col, feat) -> [2, B_H, OUT]
        yp = arr.transpose(1, 3, 4, 0, 5, 2).reshape(2, B_H, OUT)
        tv = top_idx[c] >= 0
        bv = bot_idx[c] >= 0
        y[top_idx[c][tv]] = yp[0][tv]
        y[bot_idx[c][bv]] = yp[1][bv]
    return y


def kernel(x, u, w1, b1, w2, b2, w3, b3, w4, b4, w5, b5, w6, b6, w7, b7):
    x = np.ascontiguousarray(np.asarray(x, np.float32))
    u = np.ascontiguousarray(np.asarray(u)).astype(np.int64)
    weights = [np.asarray(t, np.float32) for t in
               (w1, b1, w2, b2, w3, b3, w4, b4, w5, b5, w6, b6, w7, b7)]

    in_maps = make_in_maps(x, u, weights)
    if in_maps is None:  # quota exceeded (shouldn't happen) -> host math
        return _numpy_reference(x, u, weights)
    nc = _get_nc()
    res = run_bass_kernel_spmd(nc, in_maps, core_ids=list(range(N_CORES)))
    return unpack_outputs(res.results)
